# revision 1
# baseline (speedup 1.0000x reference)
"""Trainium2 Bass kernel v2 for bidirectional-NNF patch voting.

Key change vs v1: the GPSIMD SWDGE descriptor generation (~12ns/descriptor,
serialized on the Pool engine) was the bottleneck at ~83K 2KB-row descriptors
per core.  v2 cuts descriptors ~4x and bytes ~4x:

  - fp8_e4m3 tables (rel err 1.7e-3 vs 2e-2 budget).
  - Pass-1 (src->ref): one 9-row PATCH gather per source pixel (4608B elem,
    1 descriptor instead of 9).  The 9 sub-rows scatter to a 3x3 target
    neighborhood: dx=0 via aligned DVE adds, dx=+-1 via constant shift-matrix
    matmuls on the idle TensorEngine into PSUM row-slabs.
  - Pass-2 (ref->src): one 3-row TRIPLE gather per (ref pixel, dy)
    (1536B elem, 1 descriptor instead of 3).  Triples are grouped by target
    row-slab; host-built weight matrices (values 2.0 = wr/ws) place each
    triple's 3 columns at its (random) target x via PE matmul.
  - Cores own 24-target-row strips; slabs = (row, window) with windows
    x[0:128) (M=128) and x[128:192) (M=64).  acc[128, 48 slots, 512] f32.
  - guide numerator = S1 + 2*S2 accumulated in acc+PSUM; division and blend
    via host-precomputed winv = 1/(c1+2*c2) exactly like v1.
"""
import numpy as np
import os
import sys
import types

sys.path.insert(0, "/opt/trn_rl_repo")

import ml_dtypes

F8 = ml_dtypes.float8_e4m3fn

C, H, W = 512, 192, 192
N = H * W
ALPHA = 0.8
TAU = 0.05
NCORES = 8
RPC = H // NCORES            # target rows per core = 24
NSLOT = RPC * 2              # 48 acc slots (row x window)
P1ROWS = RPC + 2             # 26 source rows incl. dummy halo
P1IDX = 193                  # 128 (A) + 65 (B) idxs per pass-1 instruction
P1COLS = 13                  # ceil(193/16) idx columns
FCH = 6                      # fa slots per blend chunk
FCHR = 4                     # fa slots per response chunk

_D = {}

def _skip(name):
    return name in os.environ.get("V2_SKIP", "ttr,k2").split(",")


def _k(dy, dx):
    return (dy + 1) * 3 + (dx + 1)


# ---------------------------------------------------------------- host prep

def _build_tables_p1(ref8pm, nnf_sr, y0):
    """Per-core pass-1: unique patch table + idx stream per source row.

    ref8pm: [H+2, W+2, C] fp8 zero-padded pixel-major ref.
    Returns (table [VA,9C] fp8, idx_rows [26, 208] int16).
    """
    ny = nnf_sr[..., 0].astype(np.int64)
    nx = nnf_sr[..., 1].astype(np.int64)
    uid_rows = []
    for i in range(P1ROWS):
        y = y0 - 1 + i
        if 0 <= y < H:
            ua = ny[y, 0:128] * W + nx[y, 0:128]
            ub = ny[y, 127:192] * W + nx[y, 127:192]
        else:
            ua = np.full(128, -1, np.int64)
            ub = np.full(65, -1, np.int64)
        uid_rows.append((ua, ub))
    allu = np.concatenate([np.concatenate(t) for t in uid_rows])
    used = allu[allu >= 0]
    uniq, first = np.unique(used, return_index=True)
    uniq = uniq[np.argsort(first)]
    lut = np.full(N, 0, np.int32)
    lut[uniq] = np.arange(1, len(uniq) + 1, dtype=np.int32)

    VA = len(uniq) + 1
    uy, ux = uniq // W, uniq % W
    # patch = padded[uy-1+ (0..2), ux-1 + (0..2)] -> [n,3,3,C]
    win = np.lib.stride_tricks.sliding_window_view(ref8pm, (3, 3), axis=(0, 1))
    # win shape [H, W, C, 3, 3] -> index [uy, ux] gives [n, C, 3, 3]
    pat = win[uy, ux]                     # [n, C, 3, 3]
    table = np.zeros((VA, 9 * C), F8)
    table[1:] = np.ascontiguousarray(pat.transpose(0, 2, 3, 1)).reshape(len(uniq), 9 * C)

    idx_rows = np.full((P1ROWS, 208), -1, np.int16)
    for i, (ua, ub) in enumerate(uid_rows):
        ia = np.where(ua >= 0, lut[np.maximum(ua, 0)], 0)
        ib = np.where(ub >= 0, lut[np.maximum(ub, 0)], 0)
        idx_rows[i, :128] = ia
        idx_rows[i, 128:193] = ib
    return table, idx_rows


def _plan_p2(nnf_rs):
    """Global pass-2 planning (vectorized).

    Returns (per_core, nbatch[48]) where per_core[c] = (slab_off[49], uid3[],
    tx[]) sorted by slab id; nbatch = per-slab batch count (max over cores).
    """
    ty = nnf_rs[..., 0].astype(np.int64).ravel()   # target of ref pixel r
    tx = nnf_rs[..., 1].astype(np.int64).ravel()
    ry = (np.arange(N) // W)
    rx = (np.arange(N) % W)

    keys, uids, txs = [], [], []
    for dy in (-1, 0, 1):
        tgt_row = ty + dy
        src_row = ry + dy
        ok = (tgt_row >= 0) & (tgt_row < H) & (src_row >= 0) & (src_row < H)
        for wsel in (0, 1):
            wok = (tx <= 128) if wsel == 0 else (tx >= 127)
            rr = np.nonzero(ok & wok)[0]
            keys.append(tgt_row[rr] * 2 + wsel)    # global slab id (core-fused)
            uids.append(src_row[rr] * W + rx[rr])
            txs.append(tx[rr])
    key = np.concatenate(keys)
    uid = np.concatenate(uids)
    txa = np.concatenate(txs)
    order = np.argsort(key, kind="stable")
    key, uid, txa = key[order], uid[order], txa[order]
    gcounts = np.bincount(key, minlength=H * 2)    # per global slab
    goff = np.concatenate(([0], np.cumsum(gcounts)))

    per_core = []
    counts = np.zeros((NCORES, NSLOT), np.int64)
    for c in range(NCORES):
        g0, g1 = c * NSLOT, (c + 1) * NSLOT
        lo, hi = goff[g0], goff[g1]
        slab_off = goff[g0:g1 + 1] - lo
        per_core.append((slab_off, uid[lo:hi], txa[lo:hi]))
        counts[c] = gcounts[g0:g1]
    nbatch = np.maximum(1, np.ceil(counts.max(axis=0) / 128).astype(np.int64))
    assert nbatch.max() <= 4, nbatch.max()
    return per_core, nbatch


def _build_tables_p2(ref8xm, per_core_slabs, nbatch, c):
    """Per-core pass-2: triple table, idx stream, W blob.

    ref8xm: [H, W+2, C] fp8 x-padded pixel-major ref.
    Returns (table [VB,3C], idx_stream int16 [NB*128], wblob [128, NB*3*128] f8)
    """
    slab_off, uid_all, tx_all = per_core_slabs[c]
    NB = int(nbatch.sum())
    stream_uid = np.zeros(NB * 128, np.int64)
    stream_tx = np.full(NB * 128, -1, np.int64)
    bo = np.concatenate(([0], np.cumsum(nbatch * 128)))
    for s in range(NSLOT):
        n = slab_off[s + 1] - slab_off[s]
        stream_uid[bo[s]:bo[s] + n] = uid_all[slab_off[s]:slab_off[s + 1]]
        stream_tx[bo[s]:bo[s] + n] = tx_all[slab_off[s]:slab_off[s + 1]]
    used_mask = stream_tx >= 0
    used = stream_uid[used_mask]
    uniq, first = np.unique(used, return_index=True)
    uniq = uniq[np.argsort(first)]
    lut = np.full(N, 0, np.int32)
    lut[uniq] = np.arange(1, len(uniq) + 1, dtype=np.int32)
    VB = len(uniq) + 1
    uy, ux = uniq // W, uniq % W
    tri = ref8xm[uy[:, None], ux[:, None] + np.arange(3)[None, :], :]  # [n,3,C]
    table = np.zeros((VB, 3 * C), F8)
    table[1:] = tri.reshape(len(uniq), 3 * C)

    idx_stream = np.zeros(NB * 128, np.int16)
    idx_stream[used_mask] = lut[stream_uid[used_mask]]

    # W blob: per (batch, dx): [128(K) x 128(M)] entries 2.0 (vectorized)
    wblob = np.zeros((128, NB * 3 * 128), F8)
    pos_all = np.arange(NB * 128)
    batch_of = pos_all // 128
    p_of = pos_all % 128
    slab_of_batch = np.repeat(np.arange(NSLOT), nbatch)
    sl = slab_of_batch[batch_of]
    wbase = 128 * (sl % 2)
    Mw = np.where(sl % 2 == 0, 128, 64)
    for j, dx in enumerate((-1, 0, 1)):
        x = stream_tx + dx
        m = x - wbase
        ok = used_mask & (x >= 0) & (x < W) & (m >= 0) & (m < Mw)
        rr = np.nonzero(ok)[0]
        wblob[p_of[rr], (batch_of[rr] * 3 + j) * 128 + m[rr]] = 2.0
    return table, idx_stream, wblob


def _host_den(nnf_sr, nnf_rs):
    den = np.zeros(N, np.float64)
    ny = nnf_sr[..., 0].astype(np.int64)
    nx = nnf_sr[..., 1].astype(np.int64)
    sy, sx = np.meshgrid(np.arange(H), np.arange(W), indexing="ij")
    ty2 = nnf_rs[..., 0].astype(np.int64)
    tx2 = nnf_rs[..., 1].astype(np.int64)
    ry, rx = np.meshgrid(np.arange(H), np.arange(W), indexing="ij")
    for dy in (-1, 0, 1):
        for dx in (-1, 0, 1):
            t_y, t_x = sy + dy, sx + dx
            g_y, g_x = ny + dy, nx + dx
            v = ((t_y >= 0) & (t_y < H) & (t_x >= 0) & (t_x < W) &
                 (g_y >= 0) & (g_y < H) & (g_x >= 0) & (g_x < W))
            np.add.at(den, (np.where(v, t_y * W + t_x, 0)).ravel(),
                      v.ravel().astype(np.float64) * 1.0)
            t_y, t_x = ty2 + dy, tx2 + dx
            g_y, g_x = ry + dy, rx + dx
            v = ((t_y >= 0) & (t_y < H) & (t_x >= 0) & (t_x < W) &
                 (g_y >= 0) & (g_y < H) & (g_x >= 0) & (g_x < W))
            np.add.at(den, (np.where(v, t_y * W + t_x, 0)).ravel(),
                      v.ravel().astype(np.float64) * 2.0)
    winv = np.where(den == 0, 1.0, 1.0 / np.maximum(den, 1e-30))
    return winv.astype(np.float32)


def _prep(ref, f_a, nnf_sr, nnf_rs):
    ref = np.asarray(ref, np.float32)
    f_a = np.asarray(f_a, np.float32)
    nnf_sr = np.asarray(nnf_sr)
    nnf_rs = np.asarray(nnf_rs)

    refpm = np.ascontiguousarray(ref.reshape(C, N).T.reshape(H, W, C))
    ref8 = refpm.astype(F8)
    ref8pm = np.zeros((H + 2, W + 2, C), F8)
    ref8pm[1:-1, 1:-1] = ref8
    ref8xm = np.zeros((H, W + 2, C), F8)
    ref8xm[:, 1:-1] = ref8

    per_core_slabs, nbatch = _plan_p2(nnf_rs)
    NB = int(nbatch.sum())

    winv_full = _host_den(nnf_sr, nnf_rs).reshape(H, W)
    faT = f_a.reshape(C, N).T.reshape(H, W, C)

    in_maps = []
    VA_max = VB_max = 0
    tabsA, tabsB, idxAs, idxBs, wblobs = [], [], [], [], []
    for c in range(NCORES):
        y0 = c * RPC
        tA, idxA = _build_tables_p1(ref8pm, nnf_sr, y0)
        tB, idxB, wb = _build_tables_p2(ref8xm, per_core_slabs, nbatch, c)
        tabsA.append(tA); tabsB.append(tB)
        idxAs.append(idxA); idxBs.append(idxB); wblobs.append(wb)
        VA_max = max(VA_max, len(tA)); VB_max = max(VB_max, len(tB))

    # idx blob: p1 rows then p2 stream (wrapped 16p x8)
    def wrap(ix):
        return np.tile(ix.reshape(-1, 16).T, (8, 1))

    p2_n_instr = (NB + 7) // 8
    for c in range(NCORES):
        y0 = c * RPC
        TA = np.zeros((VA_max, 9 * C), F8); TA[:len(tabsA[c])] = tabsA[c]
        TB = np.zeros((VB_max, 3 * C), F8); TB[:len(tabsB[c])] = tabsB[c]
        blocks = [wrap(idxAs[c][i]) for i in range(P1ROWS)]
        blocks.append(wrap(idxBs[c].astype(np.int16)))
        idx_blob = np.ascontiguousarray(np.concatenate(blocks, axis=1))

        # fa / winv blobs in acc layout [128, 48, C] / [128, 48]
        fa_blob = np.zeros((128, NSLOT, C), np.float32)
        winv_blob = np.ones((128, NSLOT), np.float32)
        for yl in range(RPC):
            g = y0 + yl
            fa_blob[:, yl * 2, :] = faT[g, 0:128]
            fa_blob[0:64, yl * 2 + 1, :] = faT[g, 128:192]
            fa_blob[64:128, yl * 2 + 1, :] = faT[g, 128:192]  # dup for resp
            winv_blob[:, yl * 2] = winv_full[g, 0:128]
            winv_blob[0:64, yl * 2 + 1] = winv_full[g, 128:192]
        fa_flat = np.ascontiguousarray(fa_blob.reshape(128, NSLOT * C))
        in_maps.append({
            "ta": TA, "tb": TB, "idx": idx_blob.astype(np.int16),
            "wb": wblobs[c].astype(F8),
            "fa": fa_flat,
            "fab": fa_flat.astype(ml_dtypes.bfloat16),
            "winv": np.ascontiguousarray(winv_blob),
        })

    plan = dict(VA=VA_max, VB=VB_max, NB=NB, n_p2_instr=p2_n_instr,
                nbatch=tuple(int(x) for x in nbatch),
                icols=P1ROWS * P1COLS + NB * 8)
    return plan, in_maps


# ------------------------------------------------------------- device build

def _const_mats():
    """[128, 6, 128] fp8: Sm1, Sp1, L127, Sbm1, Sb0, Sbp1."""
    m = np.zeros((128, 6, 128), np.float32)
    for p in range(128):
        if p >= 1:
            m[p, 0, p - 1] = 1.0          # Sm1: target x = p-1
        if p + 1 < 128:
            m[p, 1, p + 1] = 1.0          # Sp1
    m[1, 2, 127] = 1.0                    # L127: B p=1 (x'=128) -> x=127
    for p in range(65):                   # B: x' = 127+p, m = x-128 = p+dx-1
        for j, dx in enumerate((-1, 0, 1)):
            mm = p + dx - 1
            if 0 <= mm < 64:
                m[p, 3 + j, mm] = 1.0
    return m.astype(F8)


def _build_program(plan):
    from concourse import bacc, bass, mybir, tile

    VA, VB, NB = plan["VA"], plan["VB"], plan["NB"]
    nbatch = plan["nbatch"]
    ICOLS = plan["icols"]
    nc = bacc.Bacc("TRN2", target_bir_lowering=False, debug=False,
                   num_devices=NCORES)
    dt = mybir.dt
    ta = nc.dram_tensor("ta", [VA, 9 * C], dt.float8e4, kind="ExternalInput").ap()
    tb = nc.dram_tensor("tb", [VB, 3 * C], dt.float8e4, kind="ExternalInput").ap()
    idx = nc.dram_tensor("idx", [128, ICOLS], dt.int16, kind="ExternalInput").ap()
    wbd = nc.dram_tensor("wb", [128, NB * 3 * 128], dt.float8e4, kind="ExternalInput").ap()
    fad = nc.dram_tensor("fa", [128, NSLOT * C], dt.float32, kind="ExternalInput").ap()
    fabd = nc.dram_tensor("fab", [128, NSLOT * C], dt.bfloat16, kind="ExternalInput").ap()
    wivd = nc.dram_tensor("winv", [128, NSLOT], dt.float32, kind="ExternalInput").ap()
    cstd = nc.dram_tensor("cst", [128, 6 * 128], dt.float8e4, kind="ExternalInput").ap()
    out = nc.dram_tensor("out", [128, NSLOT * C], dt.bfloat16, kind="ExternalOutput").ap()

    with tile.TileContext(nc) as tc:
        with tc.tile_pool(name="sbuf", bufs=1) as pool, \
             tc.tile_pool(name="stg1", bufs=5) as sp1, \
             tc.tile_pool(name="stg2", bufs=2) as sp2, \
             tc.tile_pool(name="wpool", bufs=3) as wpl, \
             tc.tile_pool(name="fac", bufs=2) as fap, \
             tc.tile_pool(name="dram", bufs=1, space="DRAM") as dpool, \
             tc.tile_pool(name="psum", bufs=4, space="PSUM") as psp:
            idx_sb = pool.tile([128, ICOLS], dt.int16)
            acc = pool.tile([128, NSLOT, C], dt.float32)
            winv_sb = pool.tile([128, NSLOT], dt.float32)
            cst = pool.tile([128, 6, 128], dt.float8e4)
            resp = pool.tile([128, NSLOT], dt.float32)
            wt = pool.tile([128, NSLOT], dt.float32)
            sfac = pool.tile([128, NSLOT], dt.float32)
            red1 = pool.tile([128, 2], dt.float32)
            thrb = pool.tile([128, 2], dt.float32)
            thresh = pool.tile([128, 1], dt.float32)
            ones1 = pool.tile([1, 128], dt.float32)
            flat = pool.tile([1, 256], dt.float32)
            packv = pool.tile([1, 2], dt.float32)

            nc.sync.dma_start(out=idx_sb[:], in_=idx[:])
            nc.sync.dma_start(out=winv_sb[:], in_=wivd[:])
            nc.sync.dma_start(out=cst[:], in_=cstd[:].rearrange("p (a b) -> p a b", a=6))
            nc.vector.memset(ones1[:], 1.0)
            nc.vector.memset(acc[64:128, :, :], 0.0)

            def resp_chunk(ci):
                n = min(FCHR, NSLOT - ci)
                fch = fap.tile([128, FCHR, C], dt.float32, tag="fch")
                nc.sync.dma_start(out=fch[:, :n, :], in_=fad[:, ci * C:(ci + n) * C])
                if _skip("ttr"):
                    nc.vector.tensor_mul(fch[:, :n, :], fch[:, :n, :], fch[:, :n, :])
                    nc.vector.tensor_reduce(resp[:, ci:ci + n], fch[:, :n, :],
                                            mybir.AxisListType.X, mybir.AluOpType.add)
                else:
                    for j in range(n):
                        nc.vector.tensor_tensor_reduce(
                            out=fch[:, j, :], in0=fch[:, j, :], in1=fch[:, j, :],
                            scale=1.0, scalar=0.0,
                            op0=mybir.AluOpType.mult, op1=mybir.AluOpType.add,
                            accum_out=resp[:, ci + j:ci + j + 1])

            def emit_thresh():
                nc.vector.tensor_reduce(red1[:, 0:1], resp[:],
                                        mybir.AxisListType.X, mybir.AluOpType.max)
                nc.vector.tensor_reduce(red1[:, 1:2], resp[:],
                                        mybir.AxisListType.X, mybir.AluOpType.min)
                nc.vector.tensor_scalar_mul(red1[:, 1:2], red1[:, 1:2], -1.0)
                nc.sync.dma_start(out=flat[:], in_=red1[:])
                nc.vector.tensor_reduce(
                    packv[:], flat[:].rearrange("p (k j) -> p j k", j=2),
                    mybir.AxisListType.X, mybir.AluOpType.max)
                thr2 = pool.tile([1, 2], dt.float32)
                cc_in = dpool.tile([1, 2], dt.float32)
                cc_out = dpool.tile([1, 2], dt.float32)
                if _skip("coll"):
                    nc.vector.tensor_copy(thr2[:], packv[:])
                else:
                    nc.sync.dma_start(out=cc_in[:], in_=packv[:])
                    nc.gpsimd.collective_compute(
                        "AllReduce", mybir.AluOpType.max,
                        replica_groups=[list(range(NCORES))],
                        ins=[cc_in.opt()], outs=[cc_out.opt()])
                    nc.sync.dma_start(out=thr2[:], in_=cc_out[:])
                thr_ps = psp.tile([128, 2], dt.float32, space="PSUM")
                nc.tensor.matmul(out=thr_ps[:], lhsT=ones1[:], rhs=thr2[:],
                                 start=True, stop=True)
                nc.vector.tensor_copy(thrb[:], thr_ps[:])
                tmp1 = pool.tile([128, 1], dt.float32)
                nc.vector.tensor_scalar_mul(tmp1[:], thrb[:, 0:1], TAU)
                nc.vector.scalar_tensor_tensor(
                    out=thresh[:], in0=thrb[:, 1:2], scalar=-(1.0 - TAU), in1=tmp1[:],
                    op0=mybir.AluOpType.mult, op1=mybir.AluOpType.add)
                nc.vector.tensor_tensor(wt[:], resp[:],
                                        thresh[:].to_broadcast([128, NSLOT]),
                                        mybir.AluOpType.is_gt)
                nc.vector.tensor_scalar_mul(wt[:], wt[:], ALPHA)
                tmp2 = pool.tile([128, NSLOT], dt.float32)
                nc.vector.tensor_scalar(tmp2[:], wt[:], -1.0, 1.0,
                                        mybir.AluOpType.mult, mybir.AluOpType.add)
                nc.vector.tensor_tensor(sfac[:], tmp2[:], winv_sb[:],
                                        mybir.AluOpType.mult)

            # ---------------- gather + accumulate ----------------
            p1_tiles = {}
            woff_p1 = 0

            def p1_gather(i):
                stg = sp1.tile([128, 2, 9, C], dt.float8e4, tag="p1stg")
                if _skip("p1g"):
                    p1_tiles[i] = stg
                    return
                nc.gpsimd.dma_gather(
                    out_ap=stg[:].rearrange("p a b c -> p a (b c)"), in_ap=ta,
                    idxs_ap=idx_sb[:, i * P1COLS:(i + 1) * P1COLS],
                    num_idxs=P1IDX, num_idxs_reg=P1IDX, elem_size=9 * C,
                    single_packet=False)
                p1_tiles[i] = stg

            p2_tiles = {}
            p2_woff0 = P1ROWS * P1COLS

            def p2_gather(j):
                nidx = min(1024, NB * 128 - j * 1024)
                stg = sp2.tile([128, 8, 3, C], dt.float8e4, tag="p2stg")
                if _skip("p2g"):
                    p2_tiles[j] = stg
                    return
                nc.gpsimd.dma_gather(
                    out_ap=stg[:, :(nidx + 127) // 128, :, :].rearrange(
                        "p a b c -> p a (b c)"),
                    in_ap=tb,
                    idxs_ap=idx_sb[:, p2_woff0 + j * 64:p2_woff0 + j * 64 + (nidx + 15) // 16],
                    num_idxs=nidx, num_idxs_reg=nidx, elem_size=3 * C,
                    single_packet=False)
                p2_tiles[j] = stg

            # prefetch
            for i in range(4):
                p1_gather(i)
            p2_emitted = 0
            p2_gather(0); p2_emitted = 1

            batch_cursor = 0  # global p2 batch index as slabs consume
            wmm = 0           # global W matmul index

            for yl in range(RPC):
                if yl + 4 < P1ROWS:
                    p1_gather(yl + 4)
                if yl < 12:
                    resp_chunk(yl * FCHR)
                for wsel in (0, 1):
                    s = yl * 2 + wsel
                    M = 128 if wsel == 0 else 64
                    nb = nbatch[s]
                    # ensure p2 staging for batches [batch_cursor, +nb)
                    while (batch_cursor + nb - 1) // 8 >= p2_emitted:
                        p2_gather(p2_emitted); p2_emitted += 1
                    wnmm = 3 * nb
                    wtile = wpl.tile([128, 12, 128], dt.float8e4, tag="wt")
                    nc.sync.dma_start(
                        out=wtile[:, :wnmm, :],
                        in_=wbd[:, wmm * 128:(wmm + wnmm) * 128].rearrange(
                            "p (a b) -> p a b", a=wnmm))

                    ps0 = psp.tile([128, C], dt.float32, space="PSUM", tag="ps")
                    ps = ps0[0:M, :]
                    first = True
                    # pass-1 matmuls
                    for dy in (() if _skip("p1mm") else (-1, 0, 1)):
                        i = yl - dy + 1
                        stg = p1_tiles[i]
                        if wsel == 0:
                            for cm, slot, sub in ((0, 0, _k(dy, -1)),
                                                  (1, 0, _k(dy, 1)),
                                                  (2, 1, _k(dy, -1))):
                                kk = 128 if slot == 0 else (65 if _skip("k2") else 2)
                                nc.tensor.matmul(
                                    out=ps[:], lhsT=cst[0:kk, cm, 0:M],
                                    rhs=stg[0:kk, slot, sub, :],
                                    start=first, stop=False)
                                first = False
                        else:
                            for j, dx in enumerate((-1, 0, 1)):
                                nc.tensor.matmul(
                                    out=ps[:], lhsT=cst[0:65, 3 + j, 0:M],
                                    rhs=stg[0:65, 1, _k(dy, dx), :],
                                    start=first, stop=False)
                                first = False
                    # pass-2 matmuls
                    for b in (() if _skip("p2mm") else range(nb)):
                        gj = batch_cursor + b
                        stg = p2_tiles[gj // 8]
                        slot = gj % 8
                        for dxj in range(3):
                            nc.tensor.matmul(
                                out=ps[:],
                                lhsT=wtile[:, 3 * b + dxj, 0:M],
                                rhs=stg[:, slot, dxj, :],
                                start=first, stop=(b == nb - 1 and dxj == 2))
                            first = False
                    batch_cursor += nb
                    wmm += wnmm
                    # DVE dx=0 pass-1 adds (w0 only) + PSUM merge.
                    # First write is a copy (acc starts uninitialized).
                    if wsel == 0:
                        for dy in (-1, 0, 1):
                            i = yl - dy + 1
                            stg = p1_tiles[i]
                            aslice = acc[:, s, :]
                            if dy == -1:
                                nc.vector.tensor_copy(aslice, stg[:, 0, _k(dy, 0), :])
                            else:
                                nc.vector.tensor_add(aslice, aslice,
                                                     stg[:, 0, _k(dy, 0), :])
                        nc.vector.tensor_add(acc[0:M, s, :], acc[0:M, s, :], ps[:])
                    else:
                        nc.vector.tensor_copy(acc[0:M, s, :], ps[:])
                if yl == 16:
                    emit_thresh()

            # ---------------- blend ----------------
            for ci in range(0, NSLOT, FCH):
                n = min(FCH, NSLOT - ci)
                fchb = fap.tile([128, FCH, C], dt.bfloat16, tag="fchb")
                nc.sync.dma_start(out=fchb[:, :n, :], in_=fabd[:, ci * C:(ci + n) * C])
                w_b = wt[:, ci:ci + n].unsqueeze(2).to_broadcast([128, n, C])
                s_b = sfac[:, ci:ci + n].unsqueeze(2).to_broadcast([128, n, C])
                ach = acc[:, ci:ci + n, :]
                nc.vector.tensor_tensor(fchb[:, :n, :], fchb[:, :n, :], w_b,
                                        mybir.AluOpType.mult)
                nc.vector.tensor_tensor(ach, ach, s_b, mybir.AluOpType.mult)
                nc.vector.tensor_tensor(fchb[:, :n, :], fchb[:, :n, :], ach,
                                        mybir.AluOpType.add)
                nc.sync.dma_start(out=out[:, ci * C:(ci + n) * C], in_=fchb[:, :n, :])
    nc.compile()
    return nc


def _install_ntff_hook():
    try:
        import antenv
        if "antenv.axon_hooks" not in sys.modules:
            mod = types.ModuleType("antenv.axon_hooks")
            _h = [None]
            mod.set_axon_ntff_profile_hook = lambda h: _h.__setitem__(0, h)
            mod.get_axon_ntff_profile_hook = lambda: _h[0]
            sys.modules["antenv.axon_hooks"] = mod
            antenv.axon_hooks = mod
            from trn_agent_boot.trn_boot import _ntff_profile_via_ctypes
            hook = _ntff_profile_via_ctypes('/opt/axon/libaxon_pjrt.so')
            if hook is not None:
                mod.set_axon_ntff_profile_hook(hook)
    except Exception:
        pass


def kernel(ref, f_a, nnf_sr, nnf_rs, _trace=False):
    from concourse.bass_utils import run_bass_kernel_spmd

    _install_ntff_hook()
    plan, in_maps = _prep(ref, f_a, nnf_sr, nnf_rs)
    cstm = _const_mats().reshape(128, 6 * 128)
    for m in in_maps:
        m["cst"] = cstm

    key = (plan["VA"], plan["VB"], plan["NB"], plan["nbatch"], os.environ.get("V2_SKIP", ""))
    if _D.get("key") != key:
        _D["nc"] = _build_program(plan)
        _D["key"] = key
    nc = _D["nc"]

    res = run_bass_kernel_spmd(nc, in_maps, list(range(NCORES)), trace=_trace)
    if _trace:
        _D["exec_time_ns"] = res.exec_time_ns

    outa = np.empty((1, C, H, W), np.float32)
    for c in range(NCORES):
        blob = res.results[c]["out"].astype(np.float32).reshape(128, NSLOT, C)
        y0 = c * RPC
        for yl in range(RPC):
            outa[0, :, y0 + yl, 0:128] = blob[:, yl * 2, :].T
            outa[0, :, y0 + yl, 128:192] = blob[0:64, yl * 2 + 1, :].T
    return outa

